# revision 38
# baseline (speedup 1.0000x reference)
"""Trainium2 Bass kernel for nn_BiA_Attention (deformable windowed attention).

Sharding: 8 cores = 4 batches x 2 head-groups. Core (b, g) handles batch b,
heads [4g, 4g+4) == channels [256g, 256g+256). Each core emits a partial
projection y_part = Wp[:, g-cols] @ out_g; the host sums the two partials.

Key restructurings vs the reference (all exact):
  - conv1x1 commutes with bilinear sampling: sample (Wk@x, Wv@x) tables.
  - The k/v table is written to DRAM FOUR times, tiled as 2x2 pixel blocks
    at the 4 row/col parities.  Sample offsets are bounded (<1px), so the 4
    bilinear corners of sample (sr, sc) are exactly one 2x2 block of one
    parity copy: block (sr, sc) of copy sel = 2*(y0 odd) + (x0 odd).  One
    dma_gather of 784 rows x 4KB per window fetches all corners; row/col -1
    boundaries are pre-zeroed block slots (no masks or clamps needed).
  - bilinear combine: corner A on VectorE (tensor_scalar 4x tier), corners
    B/C/D on ScalarE activations + two VectorE adds.
  - K-half transposed to channel-major via TensorE is_transpose matmuls
    (PSUM bf16) instead of xbar DMA transposes.
  - softmax: logits tiny (no max-sub); exp(bias) multiplied in bf16;
    normalization via ones-column matmul sums + reciprocal_approx_fast.
"""

import math
import numpy as np
import ml_dtypes
from contextlib import ExitStack

import concourse.bass as bass
import concourse.mybir as mybir
import concourse.tile as tile
from concourse import bacc, library_config
from concourse.bass_utils import run_bass_kernel_spmd

F32 = mybir.dt.float32
BF16 = mybir.dt.bfloat16
I16 = mybir.dt.int16
AF = mybir.ActivationFunctionType
OP = mybir.AluOpType
BF = ml_dtypes.bfloat16

B, C, H, W = 4, 512, 56, 56
HEADS, HG, STRIDE, WS, FACTOR = 8, 2, 2, 8, 2.0
HC, GC = C // HEADS, C // HG          # 64, 256
RH, RW = H // STRIDE, W // STRIDE      # 28, 28
WH, NW, NS = H // WS, (H // WS) ** 2, (H // STRIDE) * (W // STRIDE)  # 7, 49, 784
SCALE = C ** (-0.5)
BN_EPS = 1e-5
NPIX = H * W                           # 3136
NCHK = 7                               # sample chunks of 128 (last has 16)
CPAD = NCHK * 128                      # 896
GSC = FACTOR / H * (H - 1) / 2.0       # 55/56: tanh scale in pixel units

_CACHE = {}


def _chunk_pm(c):
    return 128 if c < NCHK - 1 else NS - (NCHK - 1) * 128  # 16 for c==6


def _base_const():
    # grid coords (+2 shift, harmless: floor/frac/parity are shift-invariant)
    base = np.full((128, 14, 49), 2.0, np.float32)
    for cc in range(14):
        for p in range(128):
            s = 128 * (cc % 7) + p
            if s < NS:
                v = 2.0 * (s // RW) + 2.0 if cc < 7 else 2.0 * (s % RW) + 2.0
                base[p, cc, :] = v
    return base


def _s_const():
    # sample index s = 128*mc + p, replicated along the window dim
    s = np.zeros((128, 7, 49), np.float32)
    for mc in range(7):
        for p in range(128):
            s[p, mc, :] = 128 * mc + p
    return s


def _build_program():
    nc = bacc.Bacc("TRN2", target_bir_lowering=False, num_swdge_queues=2)

    # ---------------- DRAM tensors (per-core inputs) ----------------
    x_d = nc.dram_tensor("x", [128, 4, NPIX], BF16, kind="ExternalInput")
    wqT_d = nc.dram_tensor("wqT", [128, 4, 256], BF16, kind="ExternalInput")
    wkvT_d = nc.dram_tensor("wkvT", [128, 4, 512], BF16, kind="ExternalInput")
    bq_d = nc.dram_tensor("bq", [128, 2], F32, kind="ExternalInput")
    bqs_d = nc.dram_tensor("bqs", [128, 2], F32, kind="ExternalInput")
    dws_d = nc.dram_tensor("dws", [128, 2, 27], F32, kind="ExternalInput")
    bn_d = nc.dram_tensor("bn", [128, 2, 2], F32, kind="ExternalInput")
    offwT_d = nc.dram_tensor("offwT", [128, 2, 1792], F32, kind="ExternalInput")
    wpT_d = nc.dram_tensor("wpT", [128, 2, 512], BF16, kind="ExternalInput")
    bp2_d = nc.dram_tensor("bp2", [128, 4], F32, kind="ExternalInput")
    bv_d = nc.dram_tensor("bv", [128, 2], BF16, kind="ExternalInput")
    epe_d = nc.dram_tensor("epe", [NW, 128, 7, 256], BF16, kind="ExternalInput")

    y_d = nc.dram_tensor("y", [128, 4, NPIX], F32, kind="ExternalOutput")
    # 4 parity copies, 784 blocks each, 4 pixels x 512 ch per block
    kvt4_d = nc.dram_tensor("kvt4", [4 * NS, 2048], BF16, kind="Internal")

    ident_h = nc.inline_tensor(np.eye(128, dtype=np.float32), "identc")
    identb_h = nc.inline_tensor(np.eye(128, dtype=np.float32), "identbc")
    base_h = nc.inline_tensor(_base_const(), "basec")
    sidx_h = nc.inline_tensor(_s_const(), "sidxc")

    with tile.TileContext(nc) as tc, ExitStack() as ctx:
        nc.gpsimd.load_library(library_config.mlp)

        persist = ctx.enter_context(tc.tile_pool(name="persist", bufs=1))

        # persistent tiles
        qh2 = [persist.tile([128, NPIX], BF16, name=f"qh2{t}", tag=f"qh2{t}")
               for t in range(2)]
        att = persist.tile([128, 2, NPIX], BF16)
        idxs = persist.tile([128, NW, 56], I16)
        wgt = persist.tile([128, 4, 7, 49], F32)     # wA..wD stacked
        projbias = persist.tile([128, 4], F32)
        wpT = persist.tile([128, 2, 512], BF16)
        ident = persist.tile([128, 128], F32)
        identb = persist.tile([128, 128], BF16)

        reg_ns = nc.gpsimd.to_reg(NS)
        ones64 = persist.tile([1, 64], F32)
        nc.vector.memset(ones64[:, :], 1.0)
        nc.sync.dma_start(wpT[:, :, :], wpT_d[:, :, :])
        nc.sync.dma_start(ident[:, :], ident_h[:, :])
        nc.scalar.copy(identb[:, :], ident[:, :])

        # ================= Phase A/B/C: convs + offsets + prep ============
        with tc.tile_pool(name="conv", bufs=1) as cpool, \
             tc.tile_pool(name="cpsum", bufs=1, space="PSUM") as cpsum:
            xt = cpool.tile([128, 4, NPIX], BF16)
            nc.sync.dma_start(xt[:, :, :], x_d[:, :, :])
            wkvT = cpool.tile([128, 4, 512], BF16)
            nc.sync.dma_start(wkvT[:, :, :], wkvT_d[:, :, :])
            wqT = cpool.tile([128, 4, 256], BF16)
            nc.sync.dma_start(wqT[:, :, :], wqT_d[:, :, :])
            bq = cpool.tile([128, 2], F32)
            nc.sync.dma_start(bq[:, :], bq_d[:, :])
            bqs = cpool.tile([128, 2], F32)
            nc.sync.dma_start(bqs[:, :], bqs_d[:, :])
            dws = cpool.tile([128, 2, 27], F32)
            nc.sync.dma_start(dws[:, :, :], dws_d[:, :, :])
            bn = cpool.tile([128, 2, 2], F32)
            nc.sync.dma_start(bn[:, :, :], bn_d[:, :, :])
            offwT = cpool.tile([128, 2, 1792], F32)
            nc.sync.dma_start(offwT[:, :, :], offwT_d[:, :, :])
            bp2 = cpool.tile([128, 4], F32)
            nc.sync.dma_start(bp2[:, :], bp2_d[:, :])
            bvt = cpool.tile([128, 2], BF16)
            nc.sync.dma_start(bvt[:, :], bv_d[:, :])

            # --- q conv: ch-major, into padded f32 (offset branch) + bf16 heads
            qpad = cpool.tile([128, 2, 58 * 58], F32)
            nc.vector.memset(qpad[:, :, :], 0.0)
            for t in range(2):
                for pt in range(7):
                    p0 = pt * 448
                    ps = cpsum.tile([128, 448], F32, tag="psq", bufs=2)
                    for kc in range(4):
                        nc.tensor.matmul(ps[:, :], wqT[:, kc, 128 * t:128 * (t + 1)],
                                         xt[:, kc, p0:p0 + 448], start=(kc == 0),
                                         stop=(kc == 3))
                    # f32 + bias into padded interior (8 rows of 56)
                    r0 = 8 * pt
                    dst = qpad[:, t, :].rearrange("p (r c) -> p r c", c=58)[
                        :, r0 + 1:r0 + 9, 1:57]
                    nc.scalar.activation(dst, ps[:, :].rearrange("p (a b) -> p a b", a=8),
                                         AF.Identity, bias=bq[:, t:t + 1])
                    # bf16 scaled head-pair tile
                    nc.scalar.activation(qh2[t][:, p0:p0 + 448], ps[:, :],
                                         AF.Identity, bias=bqs[:, t:t + 1],
                                         scale=SCALE)

            # --- offset branch: 3 strided dwconvs + BN + GELU ---
            c1 = cpool.tile([128, 2, 30 * 30], F32)
            nc.vector.memset(c1[:, :, :], 0.0)
            c2 = cpool.tile([128, 2, 16 * 16], F32)
            nc.vector.memset(c2[:, :, :], 0.0)
            o3 = cpool.tile([128, 2, 49], F32)

            def dwconv(dst_flat, dr0, dc0, dst_pitch, n_out, src_flat,
                       src_pitch, t, kidx):
                src3 = src_flat.rearrange("p (r c) -> p r c", c=src_pitch)
                dst3 = dst_flat.rearrange("p (r c) -> p r c", c=dst_pitch)
                dview = dst3[:, dr0:dr0 + n_out, dc0:dc0 + n_out]
                for tap in range(9):
                    dy, dx = tap // 3, tap % 3
                    sview = src3[:, dy:dy + 2 * n_out:2, dx:dx + 2 * n_out:2]
                    w = dws[:, t, 9 * kidx + tap:9 * kidx + tap + 1]
                    if tap == 0:
                        nc.vector.tensor_scalar_mul(dview, sview, w)
                    else:
                        nc.vector.scalar_tensor_tensor(dview, sview, w, dview,
                                                       op0=OP.mult, op1=OP.add)

            for t in range(2):
                dwconv(c1[:, t, :], 1, 1, 30, 28, qpad[:, t, :], 58, t, 0)
                dwconv(c2[:, t, :], 1, 1, 16, 14, c1[:, t, :], 30, t, 1)
                dwconv(o3[:, t, :], 0, 0, 7, 7, c2[:, t, :], 16, t, 2)
                nc.vector.tensor_scalar(o3[:, t, :], o3[:, t, :],
                                        bn[:, t, 0:1], bn[:, t, 1:2],
                                        op0=OP.mult, op1=OP.add)
                # tanh-form GELU (CoreSim lacks the Gelu table; |diff|<4e-4)
                gsq = cpool.tile([128, 49], F32, tag="gsq")
                nc.scalar.activation(gsq[:, :], o3[:, t, :], AF.Square)
                nc.vector.tensor_scalar(gsq[:, :], gsq[:, :], 0.044715, 1.0,
                                        op0=OP.mult, op1=OP.add)
                nc.vector.tensor_tensor(gsq[:, :], gsq[:, :], o3[:, t, :],
                                        op=OP.mult)
                nc.scalar.activation(gsq[:, :], gsq[:, :], AF.Tanh,
                                     scale=0.7978845608028654)
                nc.vector.tensor_scalar(gsq[:, :], gsq[:, :], 0.5, 0.5,
                                        op0=OP.mult, op1=OP.add)
                nc.vector.tensor_tensor(o3[:, t, :], o3[:, t, :], gsq[:, :],
                                        op=OP.mult)

            # --- offset 1x1 conv (transposed out) + tanh -> pixel coords ---
            base = cpool.tile([128, 14, 49], F32)
            nc.sync.dma_start(base[:, :, :], base_h[:, :, :])
            sidx = cpool.tile([128, 7, 49], F32)
            nc.sync.dma_start(sidx[:, :, :], sidx_h[:, :, :])
            gpix = cpool.tile([128, 14, 49], F32)
            for mc in range(14):
                pso = cpsum.tile([128, 49], F32, tag="psoff", bufs=1)
                for t in range(2):
                    nc.tensor.matmul(pso[:, :], offwT[:, t, 128 * mc:128 * (mc + 1)],
                                     o3[:, t, :], start=(t == 0), stop=(t == 1))
                tnh = cpool.tile([128, 49], F32, tag="tnh")
                nc.scalar.activation(tnh[:, :], pso[:, :], AF.Tanh)
                # gpix = tanh * GSC + base
                nc.vector.scalar_tensor_tensor(gpix[:, mc, :], tnh[:, :], GSC,
                                               base[:, mc, :], op0=OP.mult,
                                               op1=OP.add)

            # --- per-sample prep: floor, parity, corner weights, indices ---
            # floor via magic-number round-to-nearest + is_gt correction
            y0p2 = cpool.tile([128, 14, 49], F32)
            nc.vector.tensor_scalar(y0p2[:, :, :], gpix[:, :, :], 8388608.0,
                                    -8388608.0, op0=OP.add, op1=OP.add)
            corr = cpool.tile([128, 14, 49], F32)
            nc.vector.tensor_tensor(corr[:, :, :], y0p2[:, :, :], gpix[:, :, :],
                                    op=OP.is_gt)
            nc.vector.tensor_tensor(y0p2[:, :, :], y0p2[:, :, :], corr[:, :, :],
                                    op=OP.subtract)
            frac = cpool.tile([128, 14, 49], F32)
            nc.vector.tensor_tensor(frac[:, :, :], gpix[:, :, :], y0p2[:, :, :],
                                    op=OP.subtract)
            onem = cpool.tile([128, 14, 49], F32)
            nc.vector.tensor_scalar(onem[:, :, :], frac[:, :, :], -1.0, 1.0,
                                    op0=OP.mult, op1=OP.add)
            # parity selector: 1 if floor < base (odd/minus-aligned copy)
            nneg = cpool.tile([128, 14, 49], F32)
            nc.vector.tensor_tensor(nneg[:, :, :], y0p2[:, :, :], base[:, :, :],
                                    op=OP.is_lt)
            # corner weights = (y slot) * (x slot)
            nc.vector.tensor_tensor(wgt[:, 0, :, :], onem[:, 0:7, :], onem[:, 7:14, :],
                                    op=OP.mult)
            nc.vector.tensor_tensor(wgt[:, 1, :, :], onem[:, 0:7, :], frac[:, 7:14, :],
                                    op=OP.mult)
            nc.vector.tensor_tensor(wgt[:, 2, :, :], frac[:, 0:7, :], onem[:, 7:14, :],
                                    op=OP.mult)
            nc.vector.tensor_tensor(wgt[:, 3, :, :], frac[:, 0:7, :], frac[:, 7:14, :],
                                    op=OP.mult)
            # gather index = 784*sel + s,  sel = 2*nneg_y + nneg_x
            idxf = cpool.tile([128, 7, 49], F32)
            nc.vector.scalar_tensor_tensor(idxf[:, :, :], nneg[:, 0:7, :],
                                           2.0 * NS, sidx[:, :, :],
                                           op0=OP.mult, op1=OP.add)
            nc.vector.scalar_tensor_tensor(idxf[:, :, :], nneg[:, 7:14, :],
                                           float(NS), idxf[:, :, :],
                                           op0=OP.mult, op1=OP.add)

            # fold to 16-wrapped int16 index list
            for j in range(8):
                psf = cpsum.tile([16, 343], F32, tag="psf", bufs=2)
                nc.tensor.matmul(psf[:, :], ident[:, 16 * j:16 * (j + 1)],
                                 idxf[:, :, :], start=True, stop=True)
                dstT = idxs[0:16, :, j:j + 49:8].transpose([0, 2, 1])
                nc.scalar.copy(dstT, psf[:, :].rearrange("p (a b) -> p a b", a=7))
            nc.vector.memset(idxs[0:16, :, 49:56], -1)
            nc.sync.dma_start(idxs[16:32, :, :], idxs[0:16, :, :])
            nc.sync.dma_start(idxs[32:64, :, :], idxs[0:32, :, :])
            nc.sync.dma_start(idxs[64:128, :, :], idxs[0:64, :, :])

            # --- pre-zero boundary slots of the parity copies ---
            zt = cpool.tile([28, 1024], BF16)
            nc.vector.memset(zt[:, :], 0.0)
            for sel in (1, 3):     # col -1 slots: blocks (i, 0), v=0
                for u in range(2):
                    dst = bass.AP(kvt4_d[:, :].tensor, (sel * NS) * 2048 + u * 1024,
                                  [[28 * 2048, 28], [1, 512]])
                    nc.sync.dma_start(dst, zt[:, 0:512])
            for sel in (2, 3):     # row -1 slots: blocks (0, j), u=0
                dst = bass.AP(kvt4_d[:, :].tensor, (sel * NS) * 2048,
                              [[2048, 28], [1, 1024]])
                nc.sync.dma_start(dst, zt[:, :])

            # --- kv conv into SBUF, then 16 big parity-scatter DMAs ---
            kvall = cpool.tile([112, 28, 512], BF16)
            for t in range(28):
                ps = cpsum.tile([112, 512], F32, tag="pskv", bufs=2)
                for kc in range(4):
                    nc.tensor.matmul(ps[:, :], xt[:, kc, 112 * t:112 * (t + 1)],
                                     wkvT[:, kc, :], start=(kc == 0),
                                     stop=(kc == 3))
                nc.scalar.copy(kvall[:, t, :], ps[:, :])
            for cy in range(2):
                for cx in range(2):
                    sel = 2 * cy + cx
                    for a in range(2):       # row half within a row-pair
                        # tile t -> block row i = t + cy*a (skip i=28)
                        nt = 28 - (1 if (cy and a == 1) else 0)
                        u = cy if a == 0 else 1 - cy
                        for b_ in range(2):  # col parity
                            v = cx if b_ == 0 else 1 - cx
                            j0 = 0 if b_ == 0 else cx
                            n = 28 if b_ == 0 else 28 - cx
                            src = kvall[56 * a + b_: 56 * a + b_ + 2 * n - 1: 2,
                                        0:nt, :]
                            i0 = cy * a
                            dst = bass.AP(
                                kvt4_d[:, :].tensor,
                                (sel * NS + i0 * 28 + j0) * 2048
                                + (u * 2 + v) * 512,
                                [[2048, n], [28 * 2048, nt], [1, 512]])
                            nc.sync.dma_start(dst, src)

            # --- projbias = bp + Wp_g @ bv_g ---
            psb = cpsum.tile([128, 4], F32, tag="psb")
            for t4 in range(4):
                for kc2 in range(2):
                    nc.tensor.matmul(psb[:, t4:t4 + 1],
                                     wpT[:, kc2, 128 * t4:128 * (t4 + 1)],
                                     bvt[:, kc2:kc2 + 1], start=(kc2 == 0),
                                     stop=(kc2 == 1))
            nc.vector.tensor_tensor(projbias[:, :], psb[:, :], bp2[:, :], op=OP.add)

        # ===================== Phase D: window loop =======================
        with tc.tile_pool(name="gat", bufs=3) as gpool, \
             tc.tile_pool(name="cmb", bufs=3) as cmpool, \
             tc.tile_pool(name="attn", bufs=2) as apool, \
             tc.tile_pool(name="kps", bufs=2, space="PSUM") as kpsum, \
             tc.tile_pool(name="lps", bufs=2, space="PSUM") as lpsum, \
             tc.tile_pool(name="avps", bufs=1, space="PSUM") as avsum, \
             tc.tile_pool(name="nrm", bufs=2) as npool, \
             tc.tile_pool(name="proj", bufs=2) as ppool, \
             tc.tile_pool(name="ppsum", bufs=1, space="PSUM") as ppsum:
            for w in range(NW):
                wr, wc = w // WH, w % WH
                g = gpool.tile([128, 7, 2048], BF16, tag="g")
                nc.gpsimd.dma_gather(
                    out_ap=g[:, :, :], in_ap=kvt4_d[:, :],
                    idxs_ap=idxs[:, w, 0:56], num_idxs=CPAD,
                    num_idxs_reg=reg_ns, elem_size=2048, elem_step=2048,
                    queue_num=w % 2)
                epe = apool.tile([128, 7, 256], BF16, tag="epe")
                nc.sync.dma_start(epe[:, :, :], epe_d[w, :, :, :])

                kvs = cmpool.tile([128, 7, 513], BF16, tag="kvs")
                nc.vector.memset(kvs[:, :, 512:513], 1.0)
                sc = cmpool.tile([128, 2, 2, 512], BF16, tag="sc")
                for c in range(NCHK):
                    pm = _chunk_pm(c)
                    cc = c % 2
                    # corners C, D scaled on ScalarE
                    for j in range(2):
                        nc.scalar.activation(
                            sc[0:pm, cc, j, :],
                            g[0:pm, c, 512 * (j + 2):512 * (j + 3)],
                            AF.Identity, scale=wgt[0:pm, j + 2, c, w:w + 1])
                    # corners A, B fused on VectorE (base = scalar's C output)
                    nc.vector.scalar_tensor_tensor(
                        kvs[0:pm, c, 0:512], g[0:pm, c, 0:512],
                        wgt[0:pm, 0, c, w:w + 1], sc[0:pm, cc, 0, :],
                        op0=OP.mult, op1=OP.add)
                    nc.vector.scalar_tensor_tensor(
                        kvs[0:pm, c, 0:512], g[0:pm, c, 512:1024],
                        wgt[0:pm, 1, c, w:w + 1], kvs[0:pm, c, 0:512],
                        op0=OP.mult, op1=OP.add)
                    nc.vector.tensor_tensor(
                        kvs[0:pm, c, 0:512], kvs[0:pm, c, 0:512],
                        sc[0:pm, cc, 1, :], op=OP.add)

                # k transposed to channel-major via TensorE (chunk pairs share
                # one PSUM tile and one PSUM->SBUF copy)
                kT = cmpool.tile([128, 2, 7, 128], BF16, tag="kT")
                for cp in range(4):
                    c0 = 2 * cp
                    nch = 1 if cp == 3 else 2
                    kp = kpsum.tile([128, 2, 2, 128], BF16, tag="kp")
                    for dc in range(nch):
                        pm = _chunk_pm(c0 + dc)
                        for hf in range(2):
                            nc.tensor.transpose(
                                kp[:, dc, hf, 0:pm],
                                kvs[0:pm, c0 + dc, 128 * hf:128 * (hf + 1)],
                                identb[0:pm, 0:pm])
                    if nch == 2:
                        nc.scalar.copy(
                            kT[:, :, c0:c0 + 2, :].transpose([0, 2, 1, 3]),
                            kp[:, :, :, :])
                    else:
                        nc.scalar.copy(kT[:, :, 6, 0:16], kp[:, 0, :, 0:16])

                # logits + exp -> m
                m = apool.tile([128, 7, 256], BF16, tag="m")
                qview = []
                for h in range(4):
                    qview.append(
                        qh2[h // 2][:, :].rearrange("p (r c) -> p r c", c=56)[
                            64 * (h % 2):64 * (h % 2) + 64,
                            wr * 8:wr * 8 + 8, wc * 8:wc * 8 + 8])
                for h in range(4):
                    lps = lpsum.tile([128, 7, 64], F32, tag="lps")
                    for c in range(NCHK):
                        pm = _chunk_pm(c)
                        nc.tensor.matmul(
                            lps[0:pm, c, :],
                            kT[64 * (h % 2):64 * (h % 2) + 64, h // 2, c, 0:pm],
                            qview[h], start=True, stop=True)
                    nc.scalar.activation(m[:, :, 64 * h:64 * (h + 1)],
                                         lps[:, :, :], AF.Exp)
                nc.vector.tensor_tensor(m[:, :, :], m[:, :, :],
                                        epe[:, :, :], op=OP.mult)

                # attention-value matmuls (+ ones column -> softmax sums)
                av0 = avsum.tile([128, 256], F32, tag="av0")
                av1 = avsum.tile([128, 256], F32, tag="av1")
                avr = avsum.tile([64, 512], F32, tag="avr")
                avs = avr[0:1, 0:256]
                for c in range(NCHK):
                    pm = _chunk_pm(c)
                    nc.tensor.matmul(av0[:, :], kvs[0:pm, c, 256:384],
                                     m[0:pm, c, :], start=(c == 0),
                                     stop=(c == NCHK - 1))
                    nc.tensor.matmul(av1[:, :], kvs[0:pm, c, 384:512],
                                     m[0:pm, c, :], start=(c == 0),
                                     stop=(c == NCHK - 1))
                    nc.tensor.matmul(avs, kvs[0:pm, c, 512:513],
                                     m[0:pm, c, :], start=(c == 0),
                                     stop=(c == NCHK - 1))

                avsb = npool.tile([1, 256], F32, tag="avsb")
                nc.scalar.copy(avsb[:, :], avs)
                rcp = npool.tile([1, 256], F32, tag="rcp")
                nc.vector.reciprocal_approx_fast(rcp[:, :], avsb[:, :])
                rcpb_ps = avr[0:64, 256:512]
                nc.tensor.matmul(rcpb_ps, ones64[:, :], rcp[:, :],
                                 start=True, stop=True)
                rcpb = npool.tile([64, 256], F32, tag="rcpb")
                nc.scalar.copy(rcpb[:, :], rcpb_ps)

                for h in range(4):
                    avh = av0 if h < 2 else av1
                    dst = att[64 * (h % 2):64 * (h % 2) + 64, h // 2, :] \
                        .rearrange("p (r c) -> p r c", c=56)[
                            :, wr * 8:wr * 8 + 8, wc * 8:wc * 8 + 8]
                    nc.vector.tensor_tensor(
                        dst, avh[64 * (h % 2):64 * (h % 2) + 64,
                                 64 * h:64 * (h + 1)].rearrange(
                                     "p (a b) -> p a b", a=8),
                        rcpb[:, 64 * h:64 * (h + 1)].rearrange(
                            "p (a b) -> p a b", a=8),
                        op=OP.mult)

                # project the completed window-row of pixels
                if w % WH == WH - 1:
                    pt = w // WH
                    p0 = pt * 448
                    for t4 in range(4):
                        pp = ppsum.tile([128, 448], F32, tag="pp")
                        for kc2 in range(2):
                            nc.tensor.matmul(pp[:, :],
                                             wpT[:, kc2, 128 * t4:128 * (t4 + 1)],
                                             att[:, kc2, p0:p0 + 448],
                                             start=(kc2 == 0), stop=(kc2 == 1))
                        ysb = ppool.tile([128, 448], F32, tag="ysb")
                        nc.scalar.activation(ysb[:, :], pp[:, :], AF.Identity,
                                             bias=projbias[:, t4:t4 + 1])
                        nc.sync.dma_start(y_d[:, t4, p0:p0 + 448], ysb[:, :])

    nc.finalize()
    return nc


# ======================= host-side preparation ===========================

def _perm_tables():
    perm = np.arange(H * W).reshape(WH, WS, WH, WS).transpose(0, 2, 1, 3).reshape(-1)
    kr = (np.arange(NS) // RW) * STRIDE
    kc = (np.arange(NS) % RW) * STRIDE
    RI = (kr[None, :] - (np.arange(H * W) // W)[:, None] + H - 1)[perm]
    CI = (kc[None, :] - (np.arange(H * W) % W)[:, None] + W - 1)[perm]
    return RI, CI


def _host_prep(inputs):
    f = lambda a: np.ascontiguousarray(np.asarray(a, np.float32))
    x = f(inputs["x"]).reshape(B, C, NPIX)
    Wq, Wk, Wv, Wp = f(inputs["Wq"]), f(inputs["Wk"]), f(inputs["Wv"]), f(inputs["Wp"])
    bq, bk, bv, bp = f(inputs["bq"]), f(inputs["bk"]), f(inputs["bv"]), f(inputs["bp"])
    dw = [f(inputs["dw1"]), f(inputs["dw2"]), f(inputs["dw3"])]
    bng, bnb = f(inputs["bn_gamma"]), f(inputs["bn_beta"])
    off_w = f(inputs["off_w"])
    pe = f(inputs["posembed"])
    bias_scale = 1.0 / math.sqrt(1.0 + BN_EPS)

    RI, CI = _perm_tables()
    pe_exp = np.exp(pe)                       # (8, 111, 111)
    # epe per head-group: [NW, 128, 7, 4*64]
    epe_g = []
    for g in range(HG):
        Bf = pe_exp[4 * g:4 * g + 4][:, RI, CI]          # (4, 3136, 784)
        Bf = Bf.reshape(4, NW, 64, NS)
        Bp = np.zeros((4, NW, 64, CPAD), np.float32)
        Bp[..., :NS] = Bf
        e = Bp.reshape(4, NW, 64, 7, 128).transpose(1, 4, 3, 0, 2)
        epe_g.append(np.ascontiguousarray(e.reshape(NW, 128, 7, 256)).astype(BF))

    dws = np.concatenate([d.reshape(GC, 9) for d in dw], axis=1)  # (256, 27)
    dws = dws.reshape(2, 128, 27).transpose(1, 0, 2)
    bn0 = (bng * bias_scale).astype(np.float32)
    bn_t = np.stack([bn0, bnb], axis=-1).reshape(2, 128, 2).transpose(1, 0, 2)

    offp = np.zeros((1792, GC), np.float32)
    offp[0:NS] = off_w[0:NS]
    offp[CPAD:CPAD + NS] = off_w[NS:2 * NS]
    offwT = offp.T.reshape(2, 128, 1792).transpose(1, 0, 2)  # [p, kc2, m]

    in_maps = []
    for core in range(8):
        b, g = core // 2, core % 2
        sl = slice(g * GC, (g + 1) * GC)
        xt = x[b].reshape(4, 128, NPIX).transpose(1, 0, 2)
        wqT = Wq[sl, :].T.reshape(4, 128, GC).transpose(1, 0, 2)
        wkvT = np.concatenate([Wk[sl, :], Wv[sl, :]], 0).T \
            .reshape(4, 128, 512).transpose(1, 0, 2)
        bqt = bq[sl].reshape(2, 128).T
        bqst = (bq[sl] * SCALE).reshape(2, 128).T    # bias pre-scaled by SCALE
        wpTt = Wp[:, sl].T.reshape(2, 128, 512).transpose(1, 0, 2)
        bp2t = (bp if g == 0 else np.zeros_like(bp)).reshape(4, 128).T
        bvt = bv[sl].reshape(2, 128).T
        in_maps.append({
            "x": np.ascontiguousarray(xt).astype(BF),
            "wqT": np.ascontiguousarray(wqT).astype(BF),
            "wkvT": np.ascontiguousarray(wkvT).astype(BF),
            "bq": np.ascontiguousarray(bqt),
            "bqs": np.ascontiguousarray(bqst),
            "dws": np.ascontiguousarray(dws.astype(np.float32)),
            "bn": np.ascontiguousarray(bn_t.astype(np.float32)),
            "offwT": np.ascontiguousarray(offwT),
            "wpT": np.ascontiguousarray(wpTt).astype(BF),
            "bp2": np.ascontiguousarray(bp2t),
            "bv": np.ascontiguousarray(bvt).astype(BF),
            "epe": epe_g[g],
        })
    return in_maps


def _get_nc():
    if "nc" not in _CACHE:
        _CACHE["nc"] = _build_program()
    return _CACHE["nc"]


def _assemble(results):
    out = np.zeros((B, C, H, W), np.float32)
    for core, res in enumerate(results):
        b = core // 2
        y = np.asarray(res["y"])            # [128, 4, 3136]
        out[b] += y.transpose(1, 0, 2).reshape(C, H, W)
    return out


def run(inputs, trace=False, **kw):
    nc = _get_nc()
    in_maps = _host_prep(inputs)
    res = run_bass_kernel_spmd(nc, in_maps, core_ids=list(range(8)),
                               trace=trace, **kw)
    return _assemble(res.results), res


def kernel(**inputs) -> np.ndarray:
    out, _ = run(inputs, trace=False)
    return out


# revision 39
# speedup vs baseline: 1.1807x; 1.1807x over previous
"""Trainium2 Bass kernel for nn_BiA_Attention (deformable windowed attention).

Sharding: 8 cores = 4 batches x 2 head-groups. Core (b, g) handles batch b,
heads [4g, 4g+4) == channels [256g, 256g+256). Each core emits a partial
projection y_part = Wp[:, g-cols] @ out_g; the host sums the two partials.

Key restructurings vs the reference (all exact):
  - conv1x1 commutes with bilinear sampling: sample (Wk@x, Wv@x) tables.
  - The k/v table is written to DRAM FOUR times, tiled as 2x2 pixel blocks
    at the 4 row/col parities.  Sample offsets are bounded (<1px), so the 4
    bilinear corners of sample (sr, sc) are exactly one 2x2 block of one
    parity copy: block (sr, sc) of copy sel = 2*(y0 odd) + (x0 odd).  One
    dma_gather of 784 rows x 4KB per window fetches all corners; row/col -1
    boundaries are pre-zeroed block slots (no masks or clamps needed).
  - bilinear combine: corner A on VectorE (tensor_scalar 4x tier), corners
    B/C/D on ScalarE activations + two VectorE adds.
  - K-half transposed to channel-major via TensorE is_transpose matmuls
    (PSUM bf16) instead of xbar DMA transposes.
  - softmax: logits tiny (no max-sub); exp(bias) multiplied in bf16;
    normalization via ones-column matmul sums + reciprocal_approx_fast.
"""

import math
import numpy as np
import ml_dtypes
from contextlib import ExitStack

import concourse.bass as bass
import concourse.mybir as mybir
import concourse.tile as tile
from concourse import bacc, library_config
from concourse.bass_utils import run_bass_kernel_spmd

F32 = mybir.dt.float32
BF16 = mybir.dt.bfloat16
I16 = mybir.dt.int16
AF = mybir.ActivationFunctionType
OP = mybir.AluOpType
BF = ml_dtypes.bfloat16

B, C, H, W = 4, 512, 56, 56
HEADS, HG, STRIDE, WS, FACTOR = 8, 2, 2, 8, 2.0
HC, GC = C // HEADS, C // HG          # 64, 256
RH, RW = H // STRIDE, W // STRIDE      # 28, 28
WH, NW, NS = H // WS, (H // WS) ** 2, (H // STRIDE) * (W // STRIDE)  # 7, 49, 784
SCALE = C ** (-0.5)
BN_EPS = 1e-5
NPIX = H * W                           # 3136
NCHK = 7                               # sample chunks of 128 (last has 16)
CPAD = NCHK * 128                      # 896
GSC = FACTOR / H * (H - 1) / 2.0       # 55/56: tanh scale in pixel units

_CACHE = {}


def _chunk_pm(c):
    return 128 if c < NCHK - 1 else NS - (NCHK - 1) * 128  # 16 for c==6


def _base_const():
    # grid coords (+2 shift, harmless: floor/frac/parity are shift-invariant)
    base = np.full((128, 14, 49), 2.0, np.float32)
    for cc in range(14):
        for p in range(128):
            s = 128 * (cc % 7) + p
            if s < NS:
                v = 2.0 * (s // RW) + 2.0 if cc < 7 else 2.0 * (s % RW) + 2.0
                base[p, cc, :] = v
    return base


def _s_const():
    # sample index s = 128*mc + p, replicated along the window dim
    s = np.zeros((128, 7, 49), np.float32)
    for mc in range(7):
        for p in range(128):
            s[p, mc, :] = 128 * mc + p
    return s


def _build_program():
    nc = bacc.Bacc("TRN2", target_bir_lowering=False, num_swdge_queues=2)

    # ---------------- DRAM tensors (per-core inputs) ----------------
    x_d = nc.dram_tensor("x", [128, 4, NPIX], BF16, kind="ExternalInput")
    wqT_d = nc.dram_tensor("wqT", [128, 4, 256], BF16, kind="ExternalInput")
    wkvT_d = nc.dram_tensor("wkvT", [128, 4, 512], BF16, kind="ExternalInput")
    bq_d = nc.dram_tensor("bq", [128, 2], F32, kind="ExternalInput")
    bqs_d = nc.dram_tensor("bqs", [128, 2], F32, kind="ExternalInput")
    dws_d = nc.dram_tensor("dws", [128, 2, 27], F32, kind="ExternalInput")
    bn_d = nc.dram_tensor("bn", [128, 2, 2], F32, kind="ExternalInput")
    offwT_d = nc.dram_tensor("offwT", [128, 2, 1792], F32, kind="ExternalInput")
    wpT_d = nc.dram_tensor("wpT", [128, 2, 512], BF16, kind="ExternalInput")
    bp2_d = nc.dram_tensor("bp2", [128, 4], F32, kind="ExternalInput")
    bv_d = nc.dram_tensor("bv", [128, 2], BF16, kind="ExternalInput")
    epe_d = nc.dram_tensor("epe", [NW, 128, 7, 256], BF16, kind="ExternalInput")

    y_d = nc.dram_tensor("y", [128, 4, NPIX], F32, kind="ExternalOutput")
    # 4 parity copies, 784 blocks each, 4 pixels x 512 ch per block
    kvt4_d = nc.dram_tensor("kvt4", [4 * NS, 2048], BF16, kind="Internal")

    ident_h = nc.inline_tensor(np.eye(128, dtype=np.float32), "identc")
    identb_h = nc.inline_tensor(np.eye(128, dtype=np.float32), "identbc")
    base_h = nc.inline_tensor(_base_const(), "basec")
    sidx_h = nc.inline_tensor(_s_const(), "sidxc")

    with tile.TileContext(nc) as tc, ExitStack() as ctx:
        nc.gpsimd.load_library(library_config.mlp)

        persist = ctx.enter_context(tc.tile_pool(name="persist", bufs=1))

        # persistent tiles
        qh2 = [persist.tile([128, NPIX], BF16, name=f"qh2{t}", tag=f"qh2{t}")
               for t in range(2)]
        att = persist.tile([128, 2, NPIX], BF16)
        idxs = persist.tile([128, NW, 56], I16)
        wgt = persist.tile([128, 4, 7, 49], F32)     # wA..wD stacked
        projbias = persist.tile([128, 4], F32)
        wpT = persist.tile([128, 2, 512], BF16)
        ident = persist.tile([128, 128], F32)
        identb = persist.tile([128, 128], BF16)

        reg_ns = nc.gpsimd.to_reg(NS)
        ones64 = persist.tile([1, 64], F32)
        nc.vector.memset(ones64[:, :], 1.0)
        nc.sync.dma_start(wpT[:, :, :], wpT_d[:, :, :])
        nc.sync.dma_start(ident[:, :], ident_h[:, :])
        nc.scalar.copy(identb[:, :], ident[:, :])

        # ================= Phase A/B/C: convs + offsets + prep ============
        with tc.tile_pool(name="conv", bufs=1) as cpool, \
             tc.tile_pool(name="cpsum", bufs=1, space="PSUM") as cpsum:
            xt = cpool.tile([128, 4, NPIX], BF16)
            nc.sync.dma_start(xt[:, :, :], x_d[:, :, :])
            wkvT = cpool.tile([128, 4, 512], BF16)
            nc.sync.dma_start(wkvT[:, :, :], wkvT_d[:, :, :])
            wqT = cpool.tile([128, 4, 256], BF16)
            nc.sync.dma_start(wqT[:, :, :], wqT_d[:, :, :])
            bq = cpool.tile([128, 2], F32)
            nc.sync.dma_start(bq[:, :], bq_d[:, :])
            bqs = cpool.tile([128, 2], F32)
            nc.sync.dma_start(bqs[:, :], bqs_d[:, :])
            dws = cpool.tile([128, 2, 27], F32)
            nc.sync.dma_start(dws[:, :, :], dws_d[:, :, :])
            bn = cpool.tile([128, 2, 2], F32)
            nc.sync.dma_start(bn[:, :, :], bn_d[:, :, :])
            offwT = cpool.tile([128, 2, 1792], F32)
            nc.sync.dma_start(offwT[:, :, :], offwT_d[:, :, :])
            bp2 = cpool.tile([128, 4], F32)
            nc.sync.dma_start(bp2[:, :], bp2_d[:, :])
            bvt = cpool.tile([128, 2], BF16)
            nc.sync.dma_start(bvt[:, :], bv_d[:, :])

            # --- q conv: ch-major, into padded f32 (offset branch) + bf16 heads
            qpad = cpool.tile([128, 2, 58 * 58], F32)
            nc.vector.memset(qpad[:, :, :], 0.0)
            for t in range(2):
                for pt in range(7):
                    p0 = pt * 448
                    ps = cpsum.tile([128, 448], F32, tag="psq", bufs=2)
                    for kc in range(4):
                        nc.tensor.matmul(ps[:, :], wqT[:, kc, 128 * t:128 * (t + 1)],
                                         xt[:, kc, p0:p0 + 448], start=(kc == 0),
                                         stop=(kc == 3))
                    # f32 + bias into padded interior (8 rows of 56)
                    r0 = 8 * pt
                    dst = qpad[:, t, :].rearrange("p (r c) -> p r c", c=58)[
                        :, r0 + 1:r0 + 9, 1:57]
                    nc.scalar.activation(dst, ps[:, :].rearrange("p (a b) -> p a b", a=8),
                                         AF.Identity, bias=bq[:, t:t + 1])
                    # bf16 scaled head-pair tile
                    nc.scalar.activation(qh2[t][:, p0:p0 + 448], ps[:, :],
                                         AF.Identity, bias=bqs[:, t:t + 1],
                                         scale=SCALE)

            # --- offset branch: 3 strided dwconvs + BN + GELU ---
            c1 = cpool.tile([128, 2, 30 * 30], F32)
            nc.vector.memset(c1[:, :, :], 0.0)
            c2 = cpool.tile([128, 2, 16 * 16], F32)
            nc.vector.memset(c2[:, :, :], 0.0)
            o3 = cpool.tile([128, 2, 49], F32)

            def dwconv(dst_flat, dr0, dc0, dst_pitch, n_out, src_flat,
                       src_pitch, t, kidx):
                src3 = src_flat.rearrange("p (r c) -> p r c", c=src_pitch)
                dst3 = dst_flat.rearrange("p (r c) -> p r c", c=dst_pitch)
                dview = dst3[:, dr0:dr0 + n_out, dc0:dc0 + n_out]
                for tap in range(9):
                    dy, dx = tap // 3, tap % 3
                    sview = src3[:, dy:dy + 2 * n_out:2, dx:dx + 2 * n_out:2]
                    w = dws[:, t, 9 * kidx + tap:9 * kidx + tap + 1]
                    if tap == 0:
                        nc.vector.tensor_scalar_mul(dview, sview, w)
                    else:
                        nc.vector.scalar_tensor_tensor(dview, sview, w, dview,
                                                       op0=OP.mult, op1=OP.add)

            for t in range(2):
                dwconv(c1[:, t, :], 1, 1, 30, 28, qpad[:, t, :], 58, t, 0)
                dwconv(c2[:, t, :], 1, 1, 16, 14, c1[:, t, :], 30, t, 1)
                dwconv(o3[:, t, :], 0, 0, 7, 7, c2[:, t, :], 16, t, 2)
                nc.vector.tensor_scalar(o3[:, t, :], o3[:, t, :],
                                        bn[:, t, 0:1], bn[:, t, 1:2],
                                        op0=OP.mult, op1=OP.add)
                # tanh-form GELU (CoreSim lacks the Gelu table; |diff|<4e-4)
                gsq = cpool.tile([128, 49], F32, tag="gsq")
                nc.scalar.activation(gsq[:, :], o3[:, t, :], AF.Square)
                nc.vector.tensor_scalar(gsq[:, :], gsq[:, :], 0.044715, 1.0,
                                        op0=OP.mult, op1=OP.add)
                nc.vector.tensor_tensor(gsq[:, :], gsq[:, :], o3[:, t, :],
                                        op=OP.mult)
                nc.scalar.activation(gsq[:, :], gsq[:, :], AF.Tanh,
                                     scale=0.7978845608028654)
                nc.vector.tensor_scalar(gsq[:, :], gsq[:, :], 0.5, 0.5,
                                        op0=OP.mult, op1=OP.add)
                nc.vector.tensor_tensor(o3[:, t, :], o3[:, t, :], gsq[:, :],
                                        op=OP.mult)

            # --- offset 1x1 conv (transposed out) + tanh -> pixel coords ---
            base = cpool.tile([128, 14, 49], F32)
            nc.sync.dma_start(base[:, :, :], base_h[:, :, :])
            sidx = cpool.tile([128, 7, 49], F32)
            nc.sync.dma_start(sidx[:, :, :], sidx_h[:, :, :])
            gpix = cpool.tile([128, 14, 49], F32)
            for mc in range(14):
                pso = cpsum.tile([128, 49], F32, tag="psoff", bufs=1)
                for t in range(2):
                    nc.tensor.matmul(pso[:, :], offwT[:, t, 128 * mc:128 * (mc + 1)],
                                     o3[:, t, :], start=(t == 0), stop=(t == 1))
                tnh = cpool.tile([128, 49], F32, tag="tnh")
                nc.scalar.activation(tnh[:, :], pso[:, :], AF.Tanh)
                # gpix = tanh * GSC + base
                nc.vector.scalar_tensor_tensor(gpix[:, mc, :], tnh[:, :], GSC,
                                               base[:, mc, :], op0=OP.mult,
                                               op1=OP.add)

            # --- per-sample prep: floor, parity, corner weights, indices ---
            # floor via magic-number round-to-nearest + is_gt correction
            y0p2 = cpool.tile([128, 14, 49], F32)
            nc.vector.tensor_scalar(y0p2[:, :, :], gpix[:, :, :], 8388608.0,
                                    -8388608.0, op0=OP.add, op1=OP.add)
            corr = cpool.tile([128, 14, 49], F32)
            nc.vector.tensor_tensor(corr[:, :, :], y0p2[:, :, :], gpix[:, :, :],
                                    op=OP.is_gt)
            nc.vector.tensor_tensor(y0p2[:, :, :], y0p2[:, :, :], corr[:, :, :],
                                    op=OP.subtract)
            frac = cpool.tile([128, 14, 49], F32)
            nc.vector.tensor_tensor(frac[:, :, :], gpix[:, :, :], y0p2[:, :, :],
                                    op=OP.subtract)
            onem = cpool.tile([128, 14, 49], F32)
            nc.vector.tensor_scalar(onem[:, :, :], frac[:, :, :], -1.0, 1.0,
                                    op0=OP.mult, op1=OP.add)
            # parity selector: 1 if floor < base (odd/minus-aligned copy)
            nneg = cpool.tile([128, 14, 49], F32)
            nc.vector.tensor_tensor(nneg[:, :, :], y0p2[:, :, :], base[:, :, :],
                                    op=OP.is_lt)
            # corner weights = (y slot) * (x slot)
            nc.vector.tensor_tensor(wgt[:, 0, :, :], onem[:, 0:7, :], onem[:, 7:14, :],
                                    op=OP.mult)
            nc.vector.tensor_tensor(wgt[:, 1, :, :], onem[:, 0:7, :], frac[:, 7:14, :],
                                    op=OP.mult)
            nc.vector.tensor_tensor(wgt[:, 2, :, :], frac[:, 0:7, :], onem[:, 7:14, :],
                                    op=OP.mult)
            nc.vector.tensor_tensor(wgt[:, 3, :, :], frac[:, 0:7, :], frac[:, 7:14, :],
                                    op=OP.mult)
            # gather index = 784*sel + s,  sel = 2*nneg_y + nneg_x
            idxf = cpool.tile([128, 7, 49], F32)
            nc.vector.scalar_tensor_tensor(idxf[:, :, :], nneg[:, 0:7, :],
                                           2.0 * NS, sidx[:, :, :],
                                           op0=OP.mult, op1=OP.add)
            nc.vector.scalar_tensor_tensor(idxf[:, :, :], nneg[:, 7:14, :],
                                           float(NS), idxf[:, :, :],
                                           op0=OP.mult, op1=OP.add)

            # fold to 16-wrapped int16 index list
            for j in range(8):
                psf = cpsum.tile([16, 343], F32, tag="psf", bufs=2)
                nc.tensor.matmul(psf[:, :], ident[:, 16 * j:16 * (j + 1)],
                                 idxf[:, :, :], start=True, stop=True)
                dstT = idxs[0:16, :, j:j + 49:8].transpose([0, 2, 1])
                nc.scalar.copy(dstT, psf[:, :].rearrange("p (a b) -> p a b", a=7))
            nc.vector.memset(idxs[0:16, :, 49:56], -1)
            nc.sync.dma_start(idxs[16:32, :, :], idxs[0:16, :, :])
            nc.sync.dma_start(idxs[32:64, :, :], idxs[0:32, :, :])
            nc.sync.dma_start(idxs[64:128, :, :], idxs[0:64, :, :])

            # --- pre-zero boundary slots of the parity copies ---
            zt = cpool.tile([28, 1024], BF16)
            nc.vector.memset(zt[:, :], 0.0)
            for sel in (1, 3):     # col -1 slots: blocks (i, 0), v=0
                for u in range(2):
                    dst = bass.AP(kvt4_d[:, :].tensor, (sel * NS) * 2048 + u * 1024,
                                  [[28 * 2048, 28], [1, 512]])
                    nc.sync.dma_start(dst, zt[:, 0:512])
            for sel in (2, 3):     # row -1 slots: blocks (0, j), u=0
                dst = bass.AP(kvt4_d[:, :].tensor, (sel * NS) * 2048,
                              [[2048, 28], [1, 1024]])
                nc.sync.dma_start(dst, zt[:, :])

            # --- kv conv into SBUF, then 16 big parity-scatter DMAs ---
            kvall = cpool.tile([112, 28, 512], BF16)
            for t in range(28):
                ps = cpsum.tile([112, 512], F32, tag="pskv", bufs=2)
                for kc in range(4):
                    nc.tensor.matmul(ps[:, :], xt[:, kc, 112 * t:112 * (t + 1)],
                                     wkvT[:, kc, :], start=(kc == 0),
                                     stop=(kc == 3))
                nc.scalar.copy(kvall[:, t, :], ps[:, :])
            for cy in range(2):
                for cx in range(2):
                    sel = 2 * cy + cx
                    for a in range(2):       # row half within a row-pair
                        # tile t -> block row i = t + cy*a (skip i=28)
                        nt = 28 - (1 if (cy and a == 1) else 0)
                        u = cy if a == 0 else 1 - cy
                        for b_ in range(2):  # col parity
                            v = cx if b_ == 0 else 1 - cx
                            j0 = 0 if b_ == 0 else cx
                            n = 28 if b_ == 0 else 28 - cx
                            src = kvall[56 * a + b_: 56 * a + b_ + 2 * n - 1: 2,
                                        0:nt, :]
                            i0 = cy * a
                            dst = bass.AP(
                                kvt4_d[:, :].tensor,
                                (sel * NS + i0 * 28 + j0) * 2048
                                + (u * 2 + v) * 512,
                                [[2048, n], [28 * 2048, nt], [1, 512]])
                            nc.sync.dma_start(dst, src)

            # --- projbias = bp + Wp_g @ bv_g ---
            psb = cpsum.tile([128, 4], F32, tag="psb")
            for t4 in range(4):
                for kc2 in range(2):
                    nc.tensor.matmul(psb[:, t4:t4 + 1],
                                     wpT[:, kc2, 128 * t4:128 * (t4 + 1)],
                                     bvt[:, kc2:kc2 + 1], start=(kc2 == 0),
                                     stop=(kc2 == 1))
            nc.vector.tensor_tensor(projbias[:, :], psb[:, :], bp2[:, :], op=OP.add)

        # ===================== Phase D: window loop =======================
        with tc.tile_pool(name="gat", bufs=4) as gpool, \
             tc.tile_pool(name="cmb", bufs=2) as cmpool, \
             tc.tile_pool(name="attn", bufs=2) as apool, \
             tc.tile_pool(name="kps", bufs=2, space="PSUM") as kpsum, \
             tc.tile_pool(name="lps", bufs=2, space="PSUM") as lpsum, \
             tc.tile_pool(name="avps", bufs=1, space="PSUM") as avsum, \
             tc.tile_pool(name="nrm", bufs=2) as npool, \
             tc.tile_pool(name="proj", bufs=2) as ppool, \
             tc.tile_pool(name="ppsum", bufs=1, space="PSUM") as ppsum:
            for w in range(NW):
                wr, wc = w // WH, w % WH
                g = gpool.tile([128, 7, 2048], BF16, tag="g")
                nc.gpsimd.dma_gather(
                    out_ap=g[:, :, :], in_ap=kvt4_d[:, :],
                    idxs_ap=idxs[:, w, 0:56], num_idxs=CPAD,
                    num_idxs_reg=reg_ns, elem_size=2048, elem_step=2048,
                    queue_num=w % 2)
                epe = apool.tile([128, 7, 256], BF16, tag="epe")
                nc.sync.dma_start(epe[:, :, :], epe_d[w, :, :, :])

                kvs = cmpool.tile([128, 7, 513], BF16, tag="kvs")
                nc.vector.memset(kvs[:, :, 512:513], 1.0)
                sc = cmpool.tile([128, 2, 2, 512], BF16, tag="sc")
                for c in range(NCHK):
                    pm = _chunk_pm(c)
                    cc = c % 2
                    # corners C, D scaled on ScalarE
                    for j in range(2):
                        nc.scalar.activation(
                            sc[0:pm, cc, j, :],
                            g[0:pm, c, 512 * (j + 2):512 * (j + 3)],
                            AF.Identity, scale=wgt[0:pm, j + 2, c, w:w + 1])
                    # corners A, B fused on VectorE (base = scalar's C output)
                    nc.vector.scalar_tensor_tensor(
                        kvs[0:pm, c, 0:512], g[0:pm, c, 0:512],
                        wgt[0:pm, 0, c, w:w + 1], sc[0:pm, cc, 0, :],
                        op0=OP.mult, op1=OP.add)
                    nc.vector.scalar_tensor_tensor(
                        kvs[0:pm, c, 0:512], g[0:pm, c, 512:1024],
                        wgt[0:pm, 1, c, w:w + 1], kvs[0:pm, c, 0:512],
                        op0=OP.mult, op1=OP.add)
                    nc.vector.tensor_tensor(
                        kvs[0:pm, c, 0:512], kvs[0:pm, c, 0:512],
                        sc[0:pm, cc, 1, :], op=OP.add)

                # k transposed to channel-major via TensorE (chunk pairs share
                # one PSUM tile and one PSUM->SBUF copy)
                kT = cmpool.tile([128, 2, 7, 128], BF16, tag="kT")
                for cp in range(4):
                    c0 = 2 * cp
                    nch = 1 if cp == 3 else 2
                    kp = kpsum.tile([128, 2, 2, 128], BF16, tag="kp")
                    for dc in range(nch):
                        pm = _chunk_pm(c0 + dc)
                        for hf in range(2):
                            nc.tensor.transpose(
                                kp[:, dc, hf, 0:pm],
                                kvs[0:pm, c0 + dc, 128 * hf:128 * (hf + 1)],
                                identb[0:pm, 0:pm])
                    if nch == 2:
                        nc.scalar.copy(
                            kT[:, :, c0:c0 + 2, :].transpose([0, 2, 1, 3]),
                            kp[:, :, :, :])
                    else:
                        nc.scalar.copy(kT[:, :, 6, 0:16], kp[:, 0, :, 0:16])

                # logits + exp -> m
                m = apool.tile([128, 7, 256], BF16, tag="m")
                qview = []
                for h in range(4):
                    qview.append(
                        qh2[h // 2][:, :].rearrange("p (r c) -> p r c", c=56)[
                            64 * (h % 2):64 * (h % 2) + 64,
                            wr * 8:wr * 8 + 8, wc * 8:wc * 8 + 8])
                for h in range(4):
                    lps = lpsum.tile([128, 7, 64], F32, tag="lps")
                    for c in range(NCHK):
                        pm = _chunk_pm(c)
                        nc.tensor.matmul(
                            lps[0:pm, c, :],
                            kT[64 * (h % 2):64 * (h % 2) + 64, h // 2, c, 0:pm],
                            qview[h], start=True, stop=True)
                    nc.scalar.activation(m[:, :, 64 * h:64 * (h + 1)],
                                         lps[:, :, :], AF.Exp)
                nc.vector.tensor_tensor(m[:, :, :], m[:, :, :],
                                        epe[:, :, :], op=OP.mult)

                # attention-value matmuls (+ ones column -> softmax sums)
                av0 = avsum.tile([128, 256], F32, tag="av0")
                av1 = avsum.tile([128, 256], F32, tag="av1")
                avr = avsum.tile([64, 512], F32, tag="avr")
                avs = avr[0:1, 0:256]
                for c in range(NCHK):
                    pm = _chunk_pm(c)
                    nc.tensor.matmul(av0[:, :], kvs[0:pm, c, 256:384],
                                     m[0:pm, c, :], start=(c == 0),
                                     stop=(c == NCHK - 1))
                    nc.tensor.matmul(av1[:, :], kvs[0:pm, c, 384:512],
                                     m[0:pm, c, :], start=(c == 0),
                                     stop=(c == NCHK - 1))
                    nc.tensor.matmul(avs, kvs[0:pm, c, 512:513],
                                     m[0:pm, c, :], start=(c == 0),
                                     stop=(c == NCHK - 1))

                avsb = npool.tile([1, 256], F32, tag="avsb")
                nc.scalar.copy(avsb[:, :], avs)
                rcp = npool.tile([1, 256], F32, tag="rcp")
                nc.vector.reciprocal_approx_fast(rcp[:, :], avsb[:, :])
                rcpb_ps = avr[0:64, 256:512]
                nc.tensor.matmul(rcpb_ps, ones64[:, :], rcp[:, :],
                                 start=True, stop=True)
                rcpb = npool.tile([64, 256], F32, tag="rcpb")
                nc.scalar.copy(rcpb[:, :], rcpb_ps)

                for h in range(4):
                    avh = av0 if h < 2 else av1
                    dst = att[64 * (h % 2):64 * (h % 2) + 64, h // 2, :] \
                        .rearrange("p (r c) -> p r c", c=56)[
                            :, wr * 8:wr * 8 + 8, wc * 8:wc * 8 + 8]
                    nc.vector.tensor_tensor(
                        dst, avh[64 * (h % 2):64 * (h % 2) + 64,
                                 64 * h:64 * (h + 1)].rearrange(
                                     "p (a b) -> p a b", a=8),
                        rcpb[:, 64 * h:64 * (h + 1)].rearrange(
                            "p (a b) -> p a b", a=8),
                        op=OP.mult)

                # project the completed window-row of pixels
                if w % WH == WH - 1:
                    pt = w // WH
                    p0 = pt * 448
                    for t4 in range(4):
                        pp = ppsum.tile([128, 448], F32, tag="pp")
                        for kc2 in range(2):
                            nc.tensor.matmul(pp[:, :],
                                             wpT[:, kc2, 128 * t4:128 * (t4 + 1)],
                                             att[:, kc2, p0:p0 + 448],
                                             start=(kc2 == 0), stop=(kc2 == 1))
                        ysb = ppool.tile([128, 448], F32, tag="ysb")
                        nc.scalar.activation(ysb[:, :], pp[:, :], AF.Identity,
                                             bias=projbias[:, t4:t4 + 1])
                        nc.sync.dma_start(y_d[:, t4, p0:p0 + 448], ysb[:, :])

    nc.finalize()
    return nc


# ======================= host-side preparation ===========================

def _perm_tables():
    perm = np.arange(H * W).reshape(WH, WS, WH, WS).transpose(0, 2, 1, 3).reshape(-1)
    kr = (np.arange(NS) // RW) * STRIDE
    kc = (np.arange(NS) % RW) * STRIDE
    RI = (kr[None, :] - (np.arange(H * W) // W)[:, None] + H - 1)[perm]
    CI = (kc[None, :] - (np.arange(H * W) % W)[:, None] + W - 1)[perm]
    return RI, CI


def _host_prep(inputs):
    f = lambda a: np.ascontiguousarray(np.asarray(a, np.float32))
    x = f(inputs["x"]).reshape(B, C, NPIX)
    Wq, Wk, Wv, Wp = f(inputs["Wq"]), f(inputs["Wk"]), f(inputs["Wv"]), f(inputs["Wp"])
    bq, bk, bv, bp = f(inputs["bq"]), f(inputs["bk"]), f(inputs["bv"]), f(inputs["bp"])
    dw = [f(inputs["dw1"]), f(inputs["dw2"]), f(inputs["dw3"])]
    bng, bnb = f(inputs["bn_gamma"]), f(inputs["bn_beta"])
    off_w = f(inputs["off_w"])
    pe = f(inputs["posembed"])
    bias_scale = 1.0 / math.sqrt(1.0 + BN_EPS)

    RI, CI = _perm_tables()
    pe_exp = np.exp(pe)                       # (8, 111, 111)
    # epe per head-group: [NW, 128, 7, 4*64]
    epe_g = []
    for g in range(HG):
        Bf = pe_exp[4 * g:4 * g + 4][:, RI, CI]          # (4, 3136, 784)
        Bf = Bf.reshape(4, NW, 64, NS)
        Bp = np.zeros((4, NW, 64, CPAD), np.float32)
        Bp[..., :NS] = Bf
        e = Bp.reshape(4, NW, 64, 7, 128).transpose(1, 4, 3, 0, 2)
        epe_g.append(np.ascontiguousarray(e.reshape(NW, 128, 7, 256)).astype(BF))

    dws = np.concatenate([d.reshape(GC, 9) for d in dw], axis=1)  # (256, 27)
    dws = dws.reshape(2, 128, 27).transpose(1, 0, 2)
    bn0 = (bng * bias_scale).astype(np.float32)
    bn_t = np.stack([bn0, bnb], axis=-1).reshape(2, 128, 2).transpose(1, 0, 2)

    offp = np.zeros((1792, GC), np.float32)
    offp[0:NS] = off_w[0:NS]
    offp[CPAD:CPAD + NS] = off_w[NS:2 * NS]
    offwT = offp.T.reshape(2, 128, 1792).transpose(1, 0, 2)  # [p, kc2, m]

    in_maps = []
    for core in range(8):
        b, g = core // 2, core % 2
        sl = slice(g * GC, (g + 1) * GC)
        xt = x[b].reshape(4, 128, NPIX).transpose(1, 0, 2)
        wqT = Wq[sl, :].T.reshape(4, 128, GC).transpose(1, 0, 2)
        wkvT = np.concatenate([Wk[sl, :], Wv[sl, :]], 0).T \
            .reshape(4, 128, 512).transpose(1, 0, 2)
        bqt = bq[sl].reshape(2, 128).T
        bqst = (bq[sl] * SCALE).reshape(2, 128).T    # bias pre-scaled by SCALE
        wpTt = Wp[:, sl].T.reshape(2, 128, 512).transpose(1, 0, 2)
        bp2t = (bp if g == 0 else np.zeros_like(bp)).reshape(4, 128).T
        bvt = bv[sl].reshape(2, 128).T
        in_maps.append({
            "x": np.ascontiguousarray(xt).astype(BF),
            "wqT": np.ascontiguousarray(wqT).astype(BF),
            "wkvT": np.ascontiguousarray(wkvT).astype(BF),
            "bq": np.ascontiguousarray(bqt),
            "bqs": np.ascontiguousarray(bqst),
            "dws": np.ascontiguousarray(dws.astype(np.float32)),
            "bn": np.ascontiguousarray(bn_t.astype(np.float32)),
            "offwT": np.ascontiguousarray(offwT),
            "wpT": np.ascontiguousarray(wpTt).astype(BF),
            "bp2": np.ascontiguousarray(bp2t),
            "bv": np.ascontiguousarray(bvt).astype(BF),
            "epe": epe_g[g],
        })
    return in_maps


def _get_nc():
    if "nc" not in _CACHE:
        _CACHE["nc"] = _build_program()
    return _CACHE["nc"]


def _assemble(results):
    out = np.zeros((B, C, H, W), np.float32)
    for core, res in enumerate(results):
        b = core // 2
        y = np.asarray(res["y"])            # [128, 4, 3136]
        out[b] += y.transpose(1, 0, 2).reshape(C, H, W)
    return out


def run(inputs, trace=False, **kw):
    nc = _get_nc()
    in_maps = _host_prep(inputs)
    res = run_bass_kernel_spmd(nc, in_maps, core_ids=list(range(8)),
                               trace=trace, **kw)
    return _assemble(res.results), res


def kernel(**inputs) -> np.ndarray:
    out, _ = run(inputs, trace=False)
    return out
